# revision 8
# baseline (speedup 1.0000x reference)
"""CIN (xDeepFM Compressed Interaction Network) Bass/Tile kernel for TRN2.

Problem: X_0 [1024, 39, 64]; three CIN layers (units 128 each):
    had_i = outer(X_0, X_i) over channel dims, per (b, d)
    X_{i+1} = W_i @ had_i + b_i            (1x1 conv over channels)
    pooled_i = X_{i+1}.sum(d)
Output: concat(pooled_1..3) -> [1024, 384] fp32.

Pure data-parallel over batch (8 cores x 128 samples), channel-major
on-chip layout [channels, b*64+d], fp16 elementwise/matmul with fp32
PSUM accumulation. Measured ~330us HW exec, rel err 3.0e-04 (vs 539us
for the previous kernel, 1348us for the first working version).

Key design points (from perfetto/ntff trace iteration):
  * symmetric layer 1: the outer product X0 (x) X0 is computed over
    unordered pairs only (780 rows vs 1521) with W1 symmetrized on the
    host. Pairs are packed by cyclic shift t (39 odd => shifts 0..19
    cover every unordered pair exactly once, 3 shifts per 117-row
    tile): operand A is the static 3-stacked x0 copy, operand B is a
    host-rotated copy streamed per (super-tile, tile).
  * layer 2 keeps the v1 scheme (broadcast slabs of X0 rows x X1 on
    DVE, 39 x K=128 accumulating conv matmuls) but slab groups shrink
    to 2 h-values (1MB) and each group is one paired DVE tensor_mul
    (x1 repeated via a 0-step middle AP dim against both slab rows).
  * slab delivery is split: most groups by partition-broadcast DMA
    from HBM, an evenly-spread subset (offg2=4: groups 2,8,14,18, +2
    extra on the ramp-limited first super-tile) generated on the PE
    via selector matmuls and evacuated by ScalarE. SBUF->SBUF
    broadcast DMA was tried and is 10x slower (0-step source
    descriptors run at 2.7GB/s) - HBM source wins.
  * strict ensure-discipline: every DMA is issued only when its pool
    slot is already free (lookahead matched to pool bufs, cross-
    super-tile prefetch spread over layer-2 groups 12..17). This
    removed HWDGE queue-head blocking that serialized the pipeline.
  * DVE runs ONLY the had tensor_muls + pooled1 reduce; all PSUM
    evacuation on ScalarE; pooled2 falls out of the layer-3 gram for
    free via a ones row appended to the per-sample gram rhs; the gram
    itself does 2 samples per matmul with a block-diagonal rhs.
  * layer 3 never materializes its conv: pooled_3 = W3 @ Gram(X0, X2)
    (one PE transpose per 2 samples), dripped into the next super-
    tile's conv loops; last super-tile's evacs alternate Scalar/DVE
    and borrow the spare PSUM pool to shorten the tail.
  * startup: critical-path loads first, st0 slab/b1t prefetch ahead
    of deferred statics, w3/id32/b3/x0dt DMAs deferred into later
    super-tiles (the ramp is HBM-contention-bound across all 8 cores).

Env knobs: BASS_CIN_OFFG2 (PE-built slab groups, default 4),
BASS_CIN_SLABSRC (hbm|sbuf, default hbm).
"""

import os
import numpy as np

import concourse.bass as bass
import concourse.bacc as bacc
import concourse.mybir as mybir
import concourse.tile as tile
from concourse import bass_utils

F32 = mybir.dt.float32
F16 = mybir.dt.float16

B, F, D, U = 1024, 39, 64, 128
NCORES = 8
BC = B // NCORES            # 128 samples per core
BD = BC * D                 # 8192 bd-columns per core
ST = 2048                   # super-tile width
NST = BD // ST              # 4
SUB = 512                   # matmul/evac sub-tile width (one PSUM bank)
NSUB = ST // SUB            # 4
SPT = ST // D               # 32 samples per super-tile
FH = F + 1                  # gram cols per sample (39 h + ones row)

# symmetric layer-1: cyclic shifts t=0..19, packed 3 per tile
SHIFTS = [(0, 1, 2), (3, 4, 5), (6, 7, 8), (9, 10, 11),
          (12, 13, 14), (15, 16, 17), (18, 19)]
NT1 = len(SHIFTS)           # 7 tiles
THS = [len(s) * F for s in SHIFTS]   # 117 x6, 78
TH1 = max(THS)

G2H = 2                     # h-values per bc2 slab group
NG2 = (F + G2H - 1) // G2H  # 20 groups (last has 1)

_CACHE: dict = {}

def _cfg():
    offg2 = int(os.environ.get("BASS_CIN_OFFG2", "4"))   # PE-built bc2 groups
    slabsrc = os.environ.get("BASS_CIN_SLABSRC", "hbm")
    return offg2, slabsrc


def _build(offg2, slabsrc) -> bacc.Bacc:
    nc = bacc.Bacc("TRN2", target_bir_lowering=False, debug=False,
                   enable_asserts=False)
    AF = mybir.ActivationFunctionType
    dt_e = F16

    x0cp_d = nc.dram_tensor("x0cp", [F, BD], dt_e, kind="ExternalInput")
    b1t_d = nc.dram_tensor("b1t", [NST, NT1, TH1, ST], dt_e, kind="ExternalInput")
    x0q2_d = nc.dram_tensor("x0q2", [NST, F, ST], dt_e, kind="ExternalInput")
    es2_d = nc.dram_tensor("esel2", [F, F * U], dt_e, kind="ExternalInput")
    x0dt_d = nc.dram_tensor("x0dt", [2 * D, (BC // 2) * 2 * FH], dt_e, kind="ExternalInput")
    w1p_d = nc.dram_tensor("w1p", [TH1, NT1 * U], dt_e, kind="ExternalInput")
    w2p_d = nc.dram_tensor("w2p", [U, F * U], dt_e, kind="ExternalInput")
    w3p_d = nc.dram_tensor("w3p", [U, F * U], dt_e, kind="ExternalInput")
    b1_d = nc.dram_tensor("b1c", [U, 1], F32, kind="ExternalInput")
    b2_d = nc.dram_tensor("b2c", [U, 1], F32, kind="ExternalInput")
    b3_d = nc.dram_tensor("b3c", [U, 1], F32, kind="ExternalInput")  # 64*b3
    id16_d = nc.dram_tensor("id16", [U, U], dt_e, kind="ExternalInput")
    id32_d = nc.dram_tensor("id32", [U, U], F32, kind="ExternalInput")

    y_d = nc.dram_tensor("y", [BC, 3 * U], F32, kind="ExternalOutput")

    with tile.TileContext(nc) as tc:
        with (
            tc.tile_pool(name="static", bufs=1) as stat,
            tc.tile_pool(name="bb1", bufs=4) as b1p,
            tc.tile_pool(name="bc2", bufs=7) as bc2p,
            tc.tile_pool(name="had", bufs=4) as hadp,
            tc.tile_pool(name="xsb", bufs=2) as xsbp,
            tc.tile_pool(name="l3sb", bufs=4) as l3p,
            tc.tile_pool(name="ps_conv", bufs=4, space="PSUM") as ppc,
            tc.tile_pool(name="ps_tg", bufs=2, space="PSUM") as pptg,
            tc.tile_pool(name="ps_bc", bufs=2, space="PSUM") as ppbc,
        ):
            # ---- critical-path loads (sync queue) ----
            x0st3 = stat.tile([TH1, BD], dt_e)          # X0 rows tiled 3x
            nc.sync.dma_start(x0st3[0:F, :], x0cp_d[:, :])
            for j in range(1, 3):
                nc.sync.dma_start(x0st3[j * F:(j + 1) * F, :], x0st3[0:F, :])
            w1sb = stat.tile([TH1, NT1 * U], dt_e)
            nc.sync.dma_start(w1sb[:], w1p_d[:, :])

            # ---- startup prefetch for st=0: issue b1t tile 0/1 and the
            # first slab groups ahead of all deferred statics ----
            _pre_b1 = {}
            for _t in (0, 1):
                _bt = b1p.tile([TH1, ST], dt_e, tag="b1", name="b1t")
                nc.scalar.dma_start(_bt[0:THS[_t], :], b1t_d[0, _t, 0:THS[_t], :])
                _pre_b1[_t] = _bt
            _pre_bc2 = {}
            _pre_pe = []
            for _g in range(4):
                _h0 = _g * G2H
                if _g == 2:          # in pe2 for offg2>=1
                    _slab = bc2p.tile([U, G2H, ST], dt_e, tag="bc2",
                                      name="bc2pe")
                    for _i in range(G2H):
                        for _sb in range(NSUB):
                            _pre_pe.append((_slab, 0, _h0 + _i, _i, _sb))
                else:
                    _slab = bc2p.tile([U, G2H, ST], dt_e, tag="bc2",
                                      name="bc2s")
                    _src = x0q2_d[0, _h0:_h0 + G2H, :].partition_broadcast(U)
                    (nc.scalar if _g % 2 else nc.sync).dma_start(
                        _slab[:, 0:G2H, :], _src)
                _pre_bc2[_g] = _slab
            # ---- deferred statics (scalar queue; ACT idle at startup) ----
            b1sb = stat.tile([U, 1], F32)
            nc.scalar.dma_start(b1sb[:], b1_d[:, :])
            b2sb = stat.tile([U, 1], F32)
            nc.scalar.dma_start(b2sb[:], b2_d[:, :])
            esel2 = stat.tile([F, F * U], dt_e)
            nc.scalar.dma_start(esel2[:], es2_d[:, :])
            x0dt = stat.tile([2 * D, (BC // 2) * 2 * FH], dt_e)
            id16 = stat.tile([U, U], dt_e)
            nc.scalar.dma_start(id16[:], id16_d[:, :])
            w3sb = stat.tile([U, F * U], dt_e)
            id32 = stat.tile([U, U], F32)
            b3sb = stat.tile([U, 1], F32)

            pooled1 = stat.tile([U, BC], F32)
            pooled3 = stat.tile([U, BC], F32)
            g2f = stat.tile([U, FH, BC], dt_e)          # gram: [m, h', b]
            outsb = stat.tile([BC, 3 * U], F32)

            # ---- slab / B tile streaming ----
            bt1: dict = {(0, t): bt for t, bt in _pre_b1.items()}
            bc2g: dict = {(0, g): sl for g, sl in _pre_bc2.items()}
            pe_pieces: list = list(_pre_pe)
            l3_pending: list = []

            def ensure_b1(st, t, eng=None):
                if t >= NT1 or (st, t) in bt1 or st >= NST:
                    return
                bt = b1p.tile([TH1, ST], dt_e, tag="b1", name="b1t")
                th = THS[t]
                (eng or nc.sync).dma_start(bt[0:th, :], b1t_d[st, t, 0:th, :])
                bt1[(st, t)] = bt

            spreads = {0: [], 1: [10], 2: [6, 14], 3: [2, 8, 14],
                       4: [2, 8, 14, 18], 5: [2, 6, 10, 14, 18],
                       6: [2, 6, 8, 12, 14, 18]}
            pe2_std = set(spreads.get(offg2, spreads[3]))
            extra = [g for g in range(2, NG2 - 1, 2) if g not in pe2_std]
            pe2_st = [pe2_std | set(extra[:3]),
                      pe2_std, pe2_std, pe2_std]

            def gen_piece(slab, st, h, i, sb_i, reuse_w=False):
                c0 = st * ST + sb_i * SUB
                bps = ppbc.tile([U, SUB], F32, tag="bcps", name="bps")
                mm = nc.tensor.matmul(
                    bps[:], esel2[:, h * U:(h + 1) * U],
                    x0st3[0:F, c0:c0 + SUB],
                    start=True, stop=True,
                )
                if reuse_w:
                    mm.ldweights = False
                dst = slab[:, i, sb_i * SUB:(sb_i + 1) * SUB]
                nc.scalar.activation(dst, bps[:], AF.Identity)

            def ensure_bc2(st, g, eng=None):
                if g >= NG2 or (st, g) in bc2g or st >= NST:
                    return
                h0 = g * G2H
                hcnt = min(G2H, F - h0)
                if g in pe2_st[st]:
                    slab = bc2p.tile([U, G2H, ST], dt_e, tag="bc2",
                                     name="bc2pe")
                    for i in range(hcnt):
                        for sb in range(NSUB):
                            pe_pieces.append((slab, st, h0 + i, i, sb))
                else:
                    slab = bc2p.tile([U, G2H, ST], dt_e, tag="bc2",
                                     name="bc2s")
                    src = x0q2_d[st, h0:h0 + hcnt, :].partition_broadcast(U)
                    (eng or nc.sync).dma_start(slab[:, 0:hcnt, :], src)
                bc2g[(st, g)] = slab

            def drip_pe(n=2):
                prev_h = None
                for _ in range(min(n, len(pe_pieces))):
                    piece = pe_pieces.pop(0)
                    gen_piece(*piece, reuse_w=(piece[2] == prev_h))
                    prev_h = piece[2]

            def emit_l3():
                if l3_pending:
                    l3_pending.pop(0)()

            w2sb = stat.tile([U, F * U], dt_e)
            nc.sync.dma_start(w2sb[:], w2p_d[:, :])

            for st in range(NST):
                cols = slice(st * ST, (st + 1) * ST)
                if st == 1:
                    nc.scalar.dma_start(x0dt[:], x0dt_d[:, :])
                elif st == 2:
                    nc.scalar.dma_start(w3sb[:], w3p_d[:, :])
                    nc.scalar.dma_start(id32[:], id32_d[:, :])
                    nc.scalar.dma_start(b3sb[:], b3_d[:, :])

                # ---- layer 1 (symmetric shift pairs) ----
                x1ps = [ppc.tile([U, SUB], F32, tag="conv", name=f"x1ps{i}")
                        for i in range(NSUB)]
                for t in range(NT1):
                    th = THS[t]
                    bt = bt1[(st, t)]
                    had1 = hadp.tile([TH1, ST], dt_e, tag="had1")
                    nc.vector.tensor_mul(
                        had1[0:th, :], x0st3[0:th, cols], bt[0:th, :])
                    for sb_i in range(NSUB):
                        mm = nc.tensor.matmul(
                            x1ps[sb_i][:], w1sb[0:th, t * U:(t + 1) * U],
                            had1[0:th, sb_i * SUB:(sb_i + 1) * SUB],
                            start=(t == 0), stop=(t == NT1 - 1),
                        )
                        if sb_i > 0:
                            mm.ldweights = False
                    emit_l3()
                    ensure_b1(st, t + 2)
                    if t == 0:
                        ensure_bc2(st, 4)
                    elif t == 2:
                        ensure_bc2(st, 5)
                    drip_pe(3 if st == 0 else 2)
                drip_pe(3)
                x1sb = xsbp.tile([U, ST], dt_e, tag="x1")
                for sb_i in range(NSUB):
                    nc.scalar.activation(
                        x1sb[:, sb_i * SUB:(sb_i + 1) * SUB], x1ps[sb_i][:],
                        AF.Identity, bias=b1sb[:], scale=1.0)

                def red1(st=st, x1sb=x1sb):
                    nc.vector.tensor_reduce(
                        pooled1[:, st * SPT:(st + 1) * SPT],
                        x1sb[:].rearrange("p (b d) -> p b d", d=D),
                        mybir.AxisListType.X, mybir.AluOpType.add)
                l3_pending.append(red1)

                # ---- layer 2: X2 = W2 @ (X0 (x) X1) + b2 ----
                x2ps = [ppc.tile([U, SUB], F32, tag="conv", name=f"x2ps{i}")
                        for i in range(NSUB)]
                for g in range(NG2):
                    h0 = g * G2H
                    hcnt = min(G2H, F - h0)
                    slab = bc2g[(st, g)]
                    # one paired TT per slab group: x1 repeated via a
                    # 0-step middle dim against both slab rows
                    had2 = hadp.tile([U, G2H, ST], dt_e, tag="had2")
                    if hcnt == G2H:
                        nc.vector.tensor_mul(
                            had2[:],
                            x1sb[:].unsqueeze(1).broadcast_to([U, G2H, ST]),
                            slab[:])
                    else:
                        nc.vector.tensor_mul(
                            had2[:, 0, :], x1sb[:], slab[:, 0, :])
                    for i in range(hcnt):
                        h = h0 + i
                        for sb_i in range(NSUB):
                            mm = nc.tensor.matmul(
                                x2ps[sb_i][:], w2sb[:, h * U:(h + 1) * U],
                                had2[:, i, sb_i * SUB:(sb_i + 1) * SUB],
                                start=(h == 0), stop=(h == F - 1),
                            )
                            if sb_i > 0:
                                mm.ldweights = False
                        emit_l3()
                        drip_pe()
                    ensure_bc2(st, g + 6)
                    if g == 12:
                        ensure_b1(st + 1, 0)
                    elif g == 13:
                        ensure_b1(st + 1, 1)
                    elif 14 <= g <= 17:
                        ensure_bc2(st + 1, g - 14)
                drip_pe(3)
                x2sb = xsbp.tile([U, ST], dt_e, tag="x2")
                for sb_i in range(NSUB):
                    nc.scalar.activation(
                        x2sb[:, sb_i * SUB:(sb_i + 1) * SUB], x2ps[sb_i][:],
                        AF.Identity, bias=b2sb[:], scale=1.0)

                # ---- layer 3 gram pieces (dripped into next phases) ----
                last = st == NST - 1

                def queue_l3(st=st, x2sb=x2sb, last=last):
                    for s2 in range(SPT // 2):          # 2 samples / transpose
                        def piece(s2=s2, st=st, x2sb=x2sb, last=last):
                            x2t_ps = pptg.tile([U, U], dt_e, tag="tg",
                                               name="x2tps")
                            nc.tensor.transpose(
                                x2t_ps[:],
                                x2sb[:, s2 * 2 * D:(s2 + 1) * 2 * D], id16[:])
                            x2t = l3p.tile([U, U], dt_e, tag="x2t", name="x2t")
                            if last and s2 % 2:
                                nc.vector.tensor_copy(x2t[:], x2t_ps[:])
                            else:
                                nc.scalar.activation(
                                    x2t[:], x2t_ps[:], AF.Identity)
                            # block-diagonal rhs: both samples in one matmul
                            p = st * (SPT // 2) + s2
                            b0 = st * SPT + s2 * 2
                            gpool = ppbc if (last and s2 % 2) else pptg
                            gtag = "bcps" if (last and s2 % 2) else "tg"
                            g2ps = gpool.tile([U, 2 * FH], F32, tag=gtag,
                                              name="g2ps")
                            nc.tensor.matmul(
                                g2ps[:], x2t[:],
                                x0dt[:, p * 2 * FH:(p + 1) * 2 * FH],
                                start=True, stop=True,
                            )
                            src = g2ps[:].rearrange("p (b h) -> p h b", b=2)
                            if last and s2 % 2 == 0:
                                nc.vector.tensor_copy(
                                    g2f[:, :, b0:b0 + 2], src)
                            else:
                                nc.scalar.activation(
                                    g2f[:, :, b0:b0 + 2], src, AF.Identity)
                        l3_pending.append(piece)
                queue_l3()
            while l3_pending:
                l3_pending.pop(0)()

            # ---- pooled3 = W3 @ G2 + 64*b3 ----
            p3ps = ppbc.tile([U, BC], F32, tag="bcps", name="p3ps")
            for h in range(F):
                nc.tensor.matmul(
                    p3ps[:], w3sb[:, h * U:(h + 1) * U], g2f[:, h, :],
                    start=(h == 0), stop=(h == F - 1),
                )
            nc.scalar.activation(
                pooled3[:], p3ps[:], AF.Identity, bias=b3sb[:], scale=1.0)

            # ---- transpose pooled_i -> [b, o] and store ----
            trp1 = ppbc.tile([BC, U], F32, tag="bcps", name="trp1")
            nc.tensor.transpose(trp1[:], pooled1[:], id32[:])
            nc.scalar.activation(outsb[:, 0:U], trp1[:], AF.Identity)
            trp2 = ppbc.tile([BC, U], dt_e, tag="bcps", name="trp2")
            nc.tensor.transpose(trp2[:], g2f[:, F, :], id16[:])
            nc.scalar.activation(outsb[:, U:2 * U], trp2[:], AF.Identity)
            trp3 = ppbc.tile([BC, U], F32, tag="bcps", name="trp3")
            nc.tensor.transpose(trp3[:], pooled3[:], id32[:])
            nc.scalar.activation(outsb[:, 2 * U:3 * U], trp3[:], AF.Identity)
            nc.sync.dma_start(y_d[:, :], outsb[:])

    nc.compile()
    return nc


def _prep_in_maps(inputs):
    np_e = np.float16
    X0 = np.asarray(inputs["X_0"], np.float32)
    W1 = np.asarray(inputs["W1"], np.float32)
    b1 = np.asarray(inputs["b1"], np.float32)
    W2 = np.asarray(inputs["W2"], np.float32)
    b2 = np.asarray(inputs["b2"], np.float32)
    W3 = np.asarray(inputs["W3"], np.float32)
    b3 = np.asarray(inputs["b3"], np.float32)

    # symmetric W1, shift-packed
    w1r = W1.reshape(U, F, F)                    # [o, h, m]
    w1s = w1r + w1r.transpose(0, 2, 1)
    w1p = np.zeros((TH1, NT1 * U), np.float32)
    for c, shifts in enumerate(SHIFTS):
        for j, t in enumerate(shifts):
            for h in range(F):
                q = (h + t) % F
                w1p[j * F + h, c * U:(c + 1) * U] = \
                    w1r[:, h, h] if t == 0 else w1s[:, h, q]

    w2p = W2.reshape(U, F, U).transpose(2, 1, 0).reshape(U, F * U)
    w3p = W3.reshape(U, F, U).transpose(2, 1, 0).reshape(U, F * U)

    es2 = np.zeros((F, F * U), np.float32)
    for h in range(F):
        es2[h, h * U:(h + 1) * U] = 1.0

    shared = {
        "w1p": w1p.astype(np_e),
        "w2p": w2p.astype(np_e),
        "w3p": w3p.astype(np_e),
        "b1c": b1.reshape(U, 1).astype(np.float32),
        "b2c": b2.reshape(U, 1).astype(np.float32),
        "b3c": (D * b3).reshape(U, 1).astype(np.float32),
        "id16": np.eye(U, dtype=np_e),
        "id32": np.eye(U, dtype=np.float32),
        "esel2": es2.astype(np_e),
    }
    in_maps = []
    for c in range(NCORES):
        xs = X0[c * BC:(c + 1) * BC]                         # [128, 39, 64]
        x0cp = xs.transpose(1, 0, 2).reshape(F, BD)          # [h, b*64+d]
        x0dt = np.zeros((2 * D, BC // 2, 2 * FH), np.float32)
        xdh = np.ones((BC, FH, D), np.float32)
        xdh[:, :F, :] = xs.transpose(0, 1, 2).transpose(0, 1, 2)
        xdh[:, :F, :] = xs
        for p in range(BC // 2):
            x0dt[0:D, p, 0:FH] = xdh[2 * p].T
            x0dt[D:2 * D, p, FH:2 * FH] = xdh[2 * p + 1].T
        x0st = x0cp.reshape(F, NST, ST)
        x0q2 = np.ascontiguousarray(x0st.transpose(1, 0, 2))  # [st, h, c]
        b1t = np.zeros((NST, NT1, TH1, ST), np.float32)
        for ci, shifts in enumerate(SHIFTS):
            for j, t in enumerate(shifts):
                for h in range(F):
                    b1t[:, ci, j * F + h, :] = x0st[(h + t) % F]
        m = dict(shared)
        m["x0cp"] = x0cp.astype(np_e)
        m["x0dt"] = x0dt.reshape(2 * D, (BC // 2) * 2 * FH).astype(np_e)
        m["x0q2"] = x0q2.astype(np_e)
        m["b1t"] = b1t.astype(np_e)
        in_maps.append(m)
    return in_maps


def _run(inputs, trace=False, **kw):
    key = _cfg()
    if key not in _CACHE:
        _CACHE[key] = _build(*key)
    nc = _CACHE[key]
    in_maps = _prep_in_maps(inputs)
    res = bass_utils.run_bass_kernel_spmd(
        nc, in_maps, core_ids=list(range(NCORES)), trace=trace, **kw)
    y = np.concatenate([r["y"] for r in res.results], axis=0).astype(np.float32)
    return y, res


def kernel(**inputs) -> np.ndarray:
    y, _ = _run(inputs, trace=False)
    return y
